# revision 19
# baseline (speedup 1.0000x reference)
"""AlignConLoss on 8 TRN2 NeuronCores.

loss = sum_j [ logsumexp_i sim[i,j] ] - sum_j sim[j,j]
with sim = l2norm(enc2) @ l2norm(enc1).T   (B=8192, D=256, T=1)

Distribution: the BxB similarity matrix is sharded row-wise (contrast rows,
enc2) across the 8 cores.  Every core receives the full anchor matrix (enc1)
in its own HBM, so no anchor all-gather is needed.  Each core computes its
1024xB block of sim with j (anchors) on PSUM partitions and i (contrast) on
the free axis, applies exp via the ScalarE activation (folding the anchor
1/||a_j|| into the activation's per-partition scale) with a fused per-column
accumulation, and the per-column partial sums + diagonal partials are
combined across cores with a single small AllGather.

Dataflow per core:
  enc1 f32 --gpsimd cast DMA--> bf16 DRAM --xbar DMA transpose--> aT sbuf
  enc2 shard f32 --cast DMA--> sbuf, row-normalized (norms via fused
    multiply+reduce), bounced through DRAM, xbar-transposed --> cnT sbuf
  anchor row norms: each core computes its own shard's norms and they are
    shared via a tiny AllGather (4KB), 1/sqrt computed as exp(-0.5*ln(x)) so
    only one ACT table set is ever loaded.
  64 j-tiles: 4 bf16 matmuls [128x128] @ [128x512] -> psum [128,1024],
    one Exp activation with accum_out -> column partials.
  Final: AllGather of [128, 65] partials, local sum/log/subtract, and a
    [128,1] x [128,1] matmul reduces partitions to the scalar loss.
"""

import numpy as np

import concourse.bass as bass
import concourse.mybir as mybir
import concourse.tile as tile
from concourse import bacc
from concourse.bass_utils import run_bass_kernel_spmd

P = 128          # partitions
B = 8192         # batch (anchors = contrast = B)
D = 256          # embedding dim
M = 8            # cores
SH = B // M      # 1024 rows per shard
ST = SH // P     # 8 row-tiles per shard
NT = B // P      # 64 j-tiles
DH = D // P      # 2 contraction chunks of 128
IC = 512         # moving-operand free-dim chunk

F32 = mybir.dt.float32
BF16 = mybir.dt.bfloat16
AF = mybir.ActivationFunctionType
ALU = mybir.AluOpType
AX = mybir.AxisListType

REPLICAS = [list(range(M))]


def build_kernel(stage: str = "full") -> bacc.Bacc:
    nc = bacc.Bacc(
        "TRN2",
        target_bir_lowering=False,
        debug=False,
        num_devices=M,
    )
    a_ext = nc.dram_tensor("a", [B, D], F32, kind="ExternalInput").ap()
    c_ext = nc.dram_tensor("c", [SH, D], F32, kind="ExternalInput").ap()
    s_ext = nc.dram_tensor("a_s", [SH, D], F32, kind="ExternalInput").ap()
    out_ext = nc.dram_tensor("out", [1, 1], F32, kind="ExternalOutput").ap()

    with tile.TileContext(nc) as tc:
        _body(tc, nc, a_ext, c_ext, s_ext, out_ext, stage)

    nc.compile()
    return nc


def _body(tc, nc, a_ext, c_ext, s_ext, out_ext, stage="full"):
    with (
        tc.tile_pool(name="const", bufs=1) as const,
        tc.tile_pool(name="work", bufs=2) as work,
        tc.tile_pool(name="scr", bufs=3) as scr,
        tc.tile_pool(name="mm_psum", bufs=3, space="PSUM") as mm_psum,
        tc.tile_pool(name="fin_psum", bufs=1, space="PSUM") as fin_psum,
        tc.tile_pool(name="dram", bufs=1, space="DRAM") as dram,
    ):
        # ---- persistent SBUF tensors
        # aT[s][p, h, n] = bf16(a[s*SH + n, h*P + p]); one slab per shard so
        # matmuls only wait on the slab they read.
        aT = [
            const.tile([P, DH, SH], BF16, tag=f"aT{s}", name=f"aT{s}")
            for s in range(M)
        ]
        cnT = const.tile([P, DH, SH], BF16, tag="cnT")
        c_nat = const.tile([P, ST, D], BF16, tag="c_nat")
        cn_nat = const.tile([P, ST, D], BF16, tag="cn_nat")
        s_nat = const.tile([P, ST, D], BF16, tag="s_nat")
        cnorm2 = const.tile([P, ST], F32, tag="cnorm2")
        snorm2 = const.tile([P, ST], F32, tag="snorm2")
        lnc = const.tile([P, ST], F32, tag="lnc")
        lns = const.tile([P, ST], F32, tag="lns")
        rinv_c = const.tile([P, ST], F32, tag="rinv_c")
        rinv_s = const.tile([P, ST], F32, tag="rinv_s")
        anorm2 = const.tile([P, NT], F32, tag="anorm2")
        lna = const.tile([P, NT], F32, tag="lna")
        rinva = const.tile([P, NT], F32, tag="rinva")
        colpart = const.tile([P, NT], F32, tag="colpart")
        diagp = const.tile([P, ST], F32, tag="diagp")
        diagacc = const.tile([P, 1], F32, tag="diagacc")
        ones = const.tile([P, 1], F32, tag="ones")

        # ---- DRAM bounce buffers
        a_bf = [
            dram.tile([SH, D], BF16, tag=f"a_bf{s}", name=f"a_bf{s}")
            for s in range(M)
        ]
        cn_dram = dram.tile([SH, D], BF16, tag="cn_dram")
        agp_in = dram.tile([P, NT + 1], F32, tag="agp_in")
        agp_out = dram.tile([M * P, NT + 1], F32, tag="agp_out")

        nc.vector.memset(ones[:], 1.0)

        # ---- contrast shard + anchor shard natural layouts (cast DMAs).
        # These come FIRST: the norms AllGather and the cnT chain gate the
        # main loop, while the bulk anchor casts can trail behind.
        nc.gpsimd.dma_start(
            out=c_nat[:], in_=c_ext.rearrange("(t p) d -> p t d", p=P)
        )
        nc.gpsimd.dma_start(
            out=s_nat[:], in_=s_ext.rearrange("(t p) d -> p t d", p=P)
        )

        # ---- row norms (fused square+rowsum), per row-tile
        for t in range(ST):
            sq = scr.tile([P, D], BF16, tag="sq")
            nc.vector.scalar_tensor_tensor(
                out=sq[:],
                in0=c_nat[:, t],
                scalar=1.0,
                in1=c_nat[:, t],
                op0=ALU.mult,
                op1=ALU.mult,
                accum_out=cnorm2[:, t : t + 1],
            )
            sq2 = scr.tile([P, D], BF16, tag="sq")
            nc.vector.scalar_tensor_tensor(
                out=sq2[:],
                in0=s_nat[:, t],
                scalar=1.0,
                in1=s_nat[:, t],
                op0=ALU.mult,
                op1=ALU.mult,
                accum_out=snorm2[:, t : t + 1],
            )

        if stage == "prep_cs":
            chk = work.tile([P, 1], F32, tag="chk")
            nc.vector.reduce_sum(out=chk[:], in_=cnorm2[:], axis=AX.X)
            nc.vector.tensor_add(out=chk[:], in0=chk[:], in1=snorm2[:, 0:1])
            for s in range(M):
                nc.vector.tensor_add(
                    out=chk[:], in0=chk[:], in1=aT[s][:, 0, 0:1]
                )
            nc.sync.dma_start(out=out_ext, in_=chk[0:1, 0:1])
            return

        # ---- 1/sqrt via exp(-0.5 * ln x): stays in one ACT table set
        nc.scalar.activation(out=lnc[:], in_=cnorm2[:], func=AF.Ln)
        nc.scalar.activation(out=rinv_c[:], in_=lnc[:], func=AF.Exp, scale=-0.5)
        nc.scalar.activation(out=lns[:], in_=snorm2[:], func=AF.Ln)
        nc.scalar.activation(out=rinv_s[:], in_=lns[:], func=AF.Exp, scale=-0.5)

        # ---- normalize contrast rows, bounce through DRAM, transpose
        for t in range(ST):
            nc.vector.tensor_scalar_mul(
                out=cn_nat[:, t], in0=c_nat[:, t], scalar1=rinv_c[:, t : t + 1]
            )
        nc.sync.dma_start(
            out=cn_dram[:].rearrange("(t p) d -> p t d", p=P), in_=cn_nat[:]
        )
        for h in range(DH):
            nc.sync.dma_start_transpose(
                cnT[:, h, :], cn_dram[:, h * P : (h + 1) * P]
            )

        # ---- diagonal partials: sim[j,j] for this shard's j
        # diagp[p,t] = (1/||a_j||) * sum_d cn[j,d] * a_raw[j,d]
        for t in range(ST):
            sq3 = scr.tile([P, D], BF16, tag="sq")
            nc.vector.scalar_tensor_tensor(
                out=sq3[:],
                in0=cn_nat[:, t],
                scalar=rinv_s[:, t : t + 1],
                in1=s_nat[:, t],
                op0=ALU.mult,
                op1=ALU.mult,
                accum_out=diagp[:, t : t + 1],
            )
        nc.vector.reduce_sum(out=diagacc[:], in_=diagp[:], axis=AX.X)

        # ---- anchor pipeline: f32 -> bf16 (cast DMA) -> xbar transpose.
        # Anchor row norms are computed locally per slab (every core holds the
        # full anchor matrix): natural-layout reload of the bf16 slab + fused
        # square-accumulate, then 1/sqrt per slab so the exp scale for j-tile
        # jt is ready as soon as slab jt//ST has landed.
        a_nat = [
            work.tile([P, ST, D], BF16, tag="a_nat", name=f"a_nat{s}")
            for s in range(M)
        ]
        for s in range(M):
            nc.gpsimd.dma_start(
                out=a_bf[s][:], in_=a_ext[s * SH : (s + 1) * SH, :]
            )
            for h in range(DH):
                nc.sync.dma_start_transpose(
                    aT[s][:, h, :], a_bf[s][:, h * P : (h + 1) * P]
                )
            nc.sync.dma_start(
                out=a_nat[s][:],
                in_=a_bf[s][:].rearrange("(t p) d -> p t d", p=P),
            )
            for t in range(ST):
                sqa = scr.tile([P, D], BF16, tag="sq")
                nc.vector.scalar_tensor_tensor(
                    out=sqa[:],
                    in0=a_nat[s][:, t],
                    scalar=1.0,
                    in1=a_nat[s][:, t],
                    op0=ALU.mult,
                    op1=ALU.mult,
                    accum_out=anorm2[:, s * ST + t : s * ST + t + 1],
                )
            nc.scalar.activation(
                out=lna[:, s * ST : (s + 1) * ST],
                in_=anorm2[:, s * ST : (s + 1) * ST],
                func=AF.Ln,
            )
            nc.scalar.activation(
                out=rinva[:, s * ST : (s + 1) * ST],
                in_=lna[:, s * ST : (s + 1) * ST],
                func=AF.Exp,
                scale=-0.5,
            )

        if stage in ("prep", "prep_noag"):
            # drain: touch every prep result so nothing is dead-coded
            chk = work.tile([P, 1], F32, tag="chk")
            nc.vector.reduce_sum(out=chk[:], in_=rinva[:], axis=AX.X)
            nc.vector.tensor_add(out=chk[:], in0=chk[:], in1=diagacc[:])
            for s in range(M):
                nc.vector.tensor_add(
                    out=chk[:], in0=chk[:], in1=aT[s][:, 0, 0:1]
                )
            nc.vector.tensor_add(out=chk[:], in0=chk[:], in1=cnT[:, 0, 0:1])
            nc.sync.dma_start(out=out_ext, in_=chk[0:1, 0:1])
            return

        # ---- main loop: 64 j-tiles
        for jt in range(NT):
            s, jloc = jt // ST, (jt % ST) * P
            ps = mm_psum.tile([P, 2 * IC], F32, tag="mmps")
            for h in range(DH):
                w = aT[s][:, h, jloc : jloc + P]
                first, last = h == 0, h == DH - 1
                nc.tensor.matmul(
                    ps[:, 0:IC], w, cnT[:, h, 0:IC], start=first, stop=last
                )
                nc.tensor.matmul(
                    ps[:, IC : 2 * IC],
                    w,
                    cnT[:, h, IC : 2 * IC],
                    start=first,
                    stop=last,
                )
            # exp in place on PSUM: only the accum (column sums) is consumed
            nc.scalar.activation(
                out=ps[:],
                in_=ps[:],
                func=AF.Exp,
                scale=rinva[:, jt : jt + 1],
                accum_out=colpart[:, jt : jt + 1],
            )

        if stage == "nofinal":
            chk = work.tile([P, 1], F32, tag="chk")
            nc.vector.reduce_sum(out=chk[:], in_=colpart[:], axis=AX.X)
            nc.vector.tensor_add(out=chk[:], in0=chk[:], in1=diagacc[:])
            nc.sync.dma_start(out=out_ext, in_=chk[0:1, 0:1])
            return

        # ---- cross-core combine: AllGather [128, 65] -> [1024, 65]
        nc.sync.dma_start(out=agp_in[:, 0:NT], in_=colpart[:])
        nc.sync.dma_start(out=agp_in[:, NT : NT + 1], in_=diagacc[:])
        nc.gpsimd.collective_compute(
            "AllGather",
            ALU.bypass,
            replica_groups=REPLICAS,
            ins=[agp_in[:].opt()],
            outs=[agp_out[:].opt()],
        )
        gath = work.tile([P, M, NT + 1], F32, tag="gath")
        nc.sync.dma_start(
            out=gath[:], in_=agp_out[:].rearrange("(m p) f -> p m f", p=P)
        )
        S = work.tile([P, NT + 1], F32, tag="Ssum")
        nc.vector.reduce_sum(
            out=S[:], in_=gath[:].rearrange("p m f -> p f m"), axis=AX.X
        )
        lg = work.tile([P, NT], F32, tag="lg")
        lsum = work.tile([P, 1], F32, tag="lsum")
        nc.scalar.activation(
            out=lg[:], in_=S[:, 0:NT], func=AF.Ln, accum_out=lsum[:]
        )
        val = work.tile([P, 1], F32, tag="val")
        nc.vector.tensor_sub(out=val[:], in0=lsum[:], in1=S[:, NT : NT + 1])

        # ---- partition reduction to a scalar: ones.T-weighted matmul
        pres = fin_psum.tile([1, 1], F32, tag="pres")
        nc.tensor.matmul(pres[:], val[:], ones[:], start=True, stop=True)
        outsb = work.tile([1, 1], F32, tag="outsb")
        nc.vector.tensor_copy(out=outsb[:], in_=pres[:])
        nc.sync.dma_start(out=out_ext, in_=outsb[:])


_NC_CACHE = None


def _get_nc():
    global _NC_CACHE
    if _NC_CACHE is None:
        _NC_CACHE = build_kernel()
    return _NC_CACHE


def kernel(**inputs) -> np.ndarray:
    a = np.ascontiguousarray(
        np.asarray(inputs["encoder_embedding1"], dtype=np.float32)
    )
    c = np.ascontiguousarray(
        np.asarray(inputs["encoder_embedding2"], dtype=np.float32)
    )
    assert a.shape == (B, D) and c.shape == (B, D)

    nc = _get_nc()
    in_maps = [
        {
            "a": a,
            "c": c[m * SH : (m + 1) * SH],
            "a_s": a[m * SH : (m + 1) * SH],
        }
        for m in range(M)
    ]
    res = run_bass_kernel_spmd(nc, in_maps, core_ids=list(range(M)))
    return np.float32(res.results[0]["out"][0, 0])


# revision 35
# speedup vs baseline: 1.2900x; 1.2900x over previous
"""AlignConLoss on 8 TRN2 NeuronCores.

loss = sum_j [ logsumexp_i sim[i,j] ] - sum_j sim[j,j]
with sim = l2norm(enc2) @ l2norm(enc1).T   (B=8192, D=256, T=1)

Distribution: the BxB similarity matrix is sharded row-wise (contrast rows,
enc2) across the 8 cores.  Every core receives the full anchor matrix (enc1)
in its own HBM, so no anchor all-gather is needed.  Each core computes its
1024xB block of sim with j (anchors) on PSUM partitions and i (contrast) on
the free axis, applies exp via the ScalarE activation (folding the anchor
1/||a_j|| into the activation's per-partition scale) with a fused per-column
accumulation, and the per-column partial sums + diagonal partials are
combined across cores with a single small AllGather.

Dataflow per core:
  enc1 f32 --gpsimd cast DMA--> bf16 DRAM --xbar DMA transpose--> aT sbuf
  enc2 shard f32 --cast DMA--> sbuf, row-normalized (norms via fused
    multiply+reduce), bounced through DRAM, xbar-transposed --> cnT sbuf
  anchor row norms: each core computes its own shard's norms and they are
    shared via a tiny AllGather (4KB), 1/sqrt computed as exp(-0.5*ln(x)) so
    only one ACT table set is ever loaded.
  64 j-tiles: 4 bf16 matmuls [128x128] @ [128x512] -> psum [128,1024],
    one Exp activation with accum_out -> column partials.
  Final: AllGather of [128, 65] partials, local sum/log/subtract, and a
    [128,1] x [128,1] matmul reduces partitions to the scalar loss.
"""

import numpy as np

import concourse.bass as bass
import concourse.mybir as mybir
import concourse.tile as tile
from concourse import bacc
from concourse.bass_utils import run_bass_kernel_spmd
from concourse.masks import make_identity

P = 128          # partitions
B = 8192         # batch (anchors = contrast = B)
D = 256          # embedding dim
M = 8            # cores
SH = B // M      # 1024 rows per shard
ST = SH // P     # 8 row-tiles per shard
NT = B // P      # 64 j-tiles
DH = D // P      # 2 contraction chunks of 128
IC = 512         # moving-operand free-dim chunk

F32 = mybir.dt.float32
BF16 = mybir.dt.bfloat16
AF = mybir.ActivationFunctionType
ALU = mybir.AluOpType
AX = mybir.AxisListType

REPLICAS = [list(range(M))]

# Both Exp and Ln are used throughout; the default table-load pass puts them
# in different ACT table sets, which costs a ~1.3us table reload on every
# Exp<->Ln alternation.  Restrict both functions to the one set that holds
# them together so exactly one table load is ever emitted.  Set IDs are
# positional, so only membership is edited, never order.
_gat_orig = None


def _gat_shared_exp_ln(arch):
    tabs = dict(_gat_orig(arch))
    target = "natural_log_exp_and_others"
    if target in tabs:
        for name in tabs:
            if name != target:
                tabs[name] = tabs[name] - {AF.Exp, AF.Ln}
    return tabs


def _install_act_table_patch():
    global _gat_orig
    from concourse import bacc as _bacc_mod

    if _gat_orig is None:
        _gat_orig = _bacc_mod.get_activation_tables
        _bacc_mod.get_activation_tables = _gat_shared_exp_ln


def build_kernel(stage: str = "full") -> bacc.Bacc:
    _install_act_table_patch()
    nc = bacc.Bacc(
        "TRN2",
        target_bir_lowering=False,
        debug=False,
        num_devices=M,
    )
    a_ext = nc.dram_tensor("a", [B, D], F32, kind="ExternalInput").ap()
    c_ext = nc.dram_tensor("c", [SH, D], F32, kind="ExternalInput").ap()
    s_ext = nc.dram_tensor("a_s", [SH, D], F32, kind="ExternalInput").ap()
    out_ext = nc.dram_tensor("out", [1, 1], F32, kind="ExternalOutput").ap()

    with tile.TileContext(nc) as tc:
        _body(tc, nc, a_ext, c_ext, s_ext, out_ext, stage)

    nc.compile()
    return nc


def _body(tc, nc, a_ext, c_ext, s_ext, out_ext, stage="full"):
    with (
        tc.tile_pool(name="const", bufs=1) as const,
        tc.tile_pool(name="work", bufs=2) as work,
        tc.tile_pool(name="scr", bufs=3) as scr,
        tc.tile_pool(name="mm_psum", bufs=3, space="PSUM") as mm_psum,
        tc.tile_pool(name="fin_psum", bufs=1, space="PSUM") as fin_psum,
        tc.tile_pool(name="dram", bufs=1, space="DRAM") as dram,
    ):
        # ---- persistent SBUF tensors
        # aT[s][p, h, n] = bf16(a[s*SH + n, h*P + p]); one slab per shard so
        # matmuls only wait on the slab they read.
        aT = [
            const.tile([P, DH, SH], BF16, tag=f"aT{s}", name=f"aT{s}")
            for s in range(M)
        ]
        cnT = const.tile([P, DH, SH], BF16, tag="cnT")
        c_nat = const.tile([P, ST, D], BF16, tag="c_nat")
        cn_nat = const.tile([P, ST, D], BF16, tag="cn_nat")
        s_nat = const.tile([P, ST, D], BF16, tag="s_nat")
        cnorm2 = const.tile([P, ST], F32, tag="cnorm2")
        snorm2 = const.tile([P, ST], F32, tag="snorm2")
        lnc = const.tile([P, ST], F32, tag="lnc")
        lns = const.tile([P, ST], F32, tag="lns")
        rinv_c = const.tile([P, ST], F32, tag="rinv_c")
        rinv_s = const.tile([P, ST], F32, tag="rinv_s")
        anorm2 = const.tile([P, NT], F32, tag="anorm2")
        lna = const.tile([P, NT], F32, tag="lna")
        rinva = const.tile([P, NT], F32, tag="rinva")
        colpart = const.tile([P, NT], F32, tag="colpart")
        diagp = const.tile([P, ST], F32, tag="diagp")
        diagacc = const.tile([P, 1], F32, tag="diagacc")
        ones = const.tile([P, 1], F32, tag="ones")
        ident64 = const.tile([64, 64], F32, tag="ident64")
        anorm_rows = const.tile([64, P], F32, tag="anorm_rows")

        # ---- DRAM bounce buffers
        a_bf = [
            dram.tile([SH, D], BF16, tag=f"a_bf{s}", name=f"a_bf{s}")
            for s in range(M)
        ]
        cn_dram = dram.tile([SH, D], BF16, tag="cn_dram")
        agn_in = dram.tile([P, ST], F32, tag="agn_in")
        agn_out = dram.tile([M * P, ST], F32, tag="agn_out")
        agp_in = dram.tile([P, NT + 1], F32, tag="agp_in")
        agp_out = dram.tile([M * P, NT + 1], F32, tag="agp_out")

        nc.vector.memset(ones[:], 1.0)
        make_identity(nc, ident64[:])

        # ---- contrast shard + anchor shard natural layouts (cast DMAs).
        # p-major row tiling (row = 8p + r) keeps every DMA contiguous; all
        # consumers (norms, scaling, diagonal, DRAM bounce) are row-order
        # agnostic.  These come FIRST: the norms AllGather and the cnT chain
        # gate the main loop, while the bulk anchor casts trail behind.
        nc.gpsimd.dma_start(
            out=c_nat[:], in_=c_ext.rearrange("(p r) d -> p r d", p=P)
        )
        nc.gpsimd.dma_start(
            out=s_nat[:], in_=s_ext.rearrange("(p r) d -> p r d", p=P)
        )

        # ---- row norms (fused square+rowsum), per row-tile.
        # s-norms first: they feed the AllGather that gates the exp scales.
        for t in range(ST):
            sq2 = scr.tile([P, D], BF16, tag="sq")
            nc.vector.scalar_tensor_tensor(
                out=sq2[:],
                in0=s_nat[:, t],
                scalar=1.0,
                in1=s_nat[:, t],
                op0=ALU.mult,
                op1=ALU.mult,
                accum_out=snorm2[:, t : t + 1],
            )
        # share anchor shard norms: AllGather [128, 8] -> [1024, 8].
        # In p-major layout the gathered buffer is simply all B anchor norms
        # in global row order.  (The collective itself is issued below, on
        # the gpsimd queue after the first two anchor casts.)
        nc.sync.dma_start(out=agn_in[:], in_=snorm2[:])

        for t in range(ST):
            sq = scr.tile([P, D], BF16, tag="sq")
            nc.vector.scalar_tensor_tensor(
                out=sq[:],
                in0=c_nat[:, t],
                scalar=1.0,
                in1=c_nat[:, t],
                op0=ALU.mult,
                op1=ALU.mult,
                accum_out=cnorm2[:, t : t + 1],
            )

        if stage == "prep_cs":
            chk = work.tile([P, 1], F32, tag="chk")
            nc.vector.reduce_sum(out=chk[:], in_=cnorm2[:], axis=AX.X)
            nc.vector.tensor_add(out=chk[:], in0=chk[:], in1=snorm2[:, 0:1])
            for s in range(M):
                nc.vector.tensor_add(
                    out=chk[:], in0=chk[:], in1=aT[s][:, 0, 0:1]
                )
            nc.sync.dma_start(out=out_ext, in_=chk[0:1, 0:1])
            return

        # ---- 1/sqrt via exp(-0.5 * ln x): stays in one ACT table set
        nc.scalar.activation(out=lns[:], in_=snorm2[:], func=AF.Ln)
        nc.scalar.activation(out=rinv_s[:], in_=lns[:], func=AF.Exp, scale=-0.5)
        nc.scalar.activation(out=lnc[:], in_=cnorm2[:], func=AF.Ln)
        nc.scalar.activation(out=rinv_c[:], in_=lnc[:], func=AF.Exp, scale=-0.5)

        # ---- anchor pipeline head: first two slabs cast + transposed, then
        # the norms collective on the gpsimd queue (so its wait stalls only
        # the remaining cast issues, not the head of the pipeline).
        for s in range(2):
            nc.gpsimd.dma_start(
                out=a_bf[s][:], in_=a_ext[s * SH : (s + 1) * SH, :]
            )
            for h in range(DH):
                nc.sync.dma_start_transpose(
                    aT[s][:, h, :], a_bf[s][:, h * P : (h + 1) * P]
                )
        nc.gpsimd.collective_compute(
            "AllGather",
            ALU.bypass,
            replica_groups=REPLICAS,
            ins=[agn_in[:].opt()],
            outs=[agn_out[:].opt()],
        )

        # AG readback + anchor 1/sqrt: gated on the collective, so placed
        # after the contrast-side rinvs on the scalar queue.  The gathered
        # buffer is B norms in row order; the exp scale needs them indexed
        # [p, jt] with j = jt*128 + p, i.e. the transpose of the row-order
        # view [jt, p] -- a single 32KB PE transpose.
        nc.scalar.dma_start(
            out=anorm_rows[:],
            in_=agn_out[:].rearrange("(j q) r -> j (q r)", j=NT),
        )
        anorm_ps = fin_psum.tile([P, NT], F32, tag="anorm_ps")
        nc.tensor.transpose(anorm_ps[:], anorm_rows[:], ident64[:])
        nc.vector.tensor_copy(out=anorm2[:], in_=anorm_ps[:])
        nc.scalar.activation(out=lna[:], in_=anorm2[:], func=AF.Ln)
        nc.scalar.activation(out=rinva[:], in_=lna[:], func=AF.Exp, scale=-0.5)

        # ---- normalize contrast rows, bounce through DRAM, transpose
        for t in range(ST):
            nc.vector.tensor_scalar_mul(
                out=cn_nat[:, t], in0=c_nat[:, t], scalar1=rinv_c[:, t : t + 1]
            )
        nc.sync.dma_start(
            out=cn_dram[:].rearrange("(p r) d -> p r d", p=P), in_=cn_nat[:]
        )
        for h in range(DH):
            nc.sync.dma_start_transpose(
                cnT[:, h, :], cn_dram[:, h * P : (h + 1) * P]
            )

        # ---- diagonal partials: sim[j,j] for this shard's j
        # diagp[p,t] = (1/||a_j||) * sum_d cn[j,d] * a_raw[j,d]
        for t in range(ST):
            sq3 = scr.tile([P, D], BF16, tag="sq")
            nc.vector.scalar_tensor_tensor(
                out=sq3[:],
                in0=cn_nat[:, t],
                scalar=rinv_s[:, t : t + 1],
                in1=s_nat[:, t],
                op0=ALU.mult,
                op1=ALU.mult,
                accum_out=diagp[:, t : t + 1],
            )
        nc.vector.reduce_sum(out=diagacc[:], in_=diagp[:], axis=AX.X)

        # ---- anchor pipeline tail: remaining slabs
        for s in range(2, M):
            nc.gpsimd.dma_start(
                out=a_bf[s][:], in_=a_ext[s * SH : (s + 1) * SH, :]
            )
            for h in range(DH):
                nc.sync.dma_start_transpose(
                    aT[s][:, h, :], a_bf[s][:, h * P : (h + 1) * P]
                )

        if stage in ("prep", "prep_noag"):
            # drain: touch every prep result so nothing is dead-coded
            chk = work.tile([P, 1], F32, tag="chk")
            nc.vector.reduce_sum(out=chk[:], in_=rinva[:], axis=AX.X)
            nc.vector.tensor_add(out=chk[:], in0=chk[:], in1=diagacc[:])
            for s in range(M):
                nc.vector.tensor_add(
                    out=chk[:], in0=chk[:], in1=aT[s][:, 0, 0:1]
                )
            nc.vector.tensor_add(out=chk[:], in0=chk[:], in1=cnT[:, 0, 0:1])
            nc.sync.dma_start(out=out_ext, in_=chk[0:1, 0:1])
            return

        # ---- main loop: 64 j-tiles
        for jt in range(NT):
            s, jloc = jt // ST, (jt % ST) * P
            ps = mm_psum.tile([P, 2 * IC], F32, tag="mmps")
            for h in range(DH):
                w = aT[s][:, h, jloc : jloc + P]
                first, last = h == 0, h == DH - 1
                nc.tensor.matmul(
                    ps[:, 0:IC], w, cnT[:, h, 0:IC], start=first, stop=last
                )
                nc.tensor.matmul(
                    ps[:, IC : 2 * IC],
                    w,
                    cnT[:, h, IC : 2 * IC],
                    start=first,
                    stop=last,
                )
            # exp in place on PSUM: only the accum (column sums) is consumed
            nc.scalar.activation(
                out=ps[:],
                in_=ps[:],
                func=AF.Exp,
                scale=rinva[:, jt : jt + 1],
                accum_out=colpart[:, jt : jt + 1],
            )

        if stage == "nofinal":
            chk = work.tile([P, 1], F32, tag="chk")
            nc.vector.reduce_sum(out=chk[:], in_=colpart[:], axis=AX.X)
            nc.vector.tensor_add(out=chk[:], in0=chk[:], in1=diagacc[:])
            nc.sync.dma_start(out=out_ext, in_=chk[0:1, 0:1])
            return

        # ---- cross-core combine: AllGather [128, 65] -> [1024, 65]
        nc.sync.dma_start(out=agp_in[:, 0:NT], in_=colpart[:])
        nc.sync.dma_start(out=agp_in[:, NT : NT + 1], in_=diagacc[:])
        nc.gpsimd.collective_compute(
            "AllGather",
            ALU.bypass,
            replica_groups=REPLICAS,
            ins=[agp_in[:].opt()],
            outs=[agp_out[:].opt()],
        )
        gath = work.tile([P, M, NT + 1], F32, tag="gath")
        nc.sync.dma_start(
            out=gath[:], in_=agp_out[:].rearrange("(m p) f -> p m f", p=P)
        )
        S = work.tile([P, NT + 1], F32, tag="Ssum")
        nc.vector.reduce_sum(
            out=S[:], in_=gath[:].rearrange("p m f -> p f m"), axis=AX.X
        )
        lg = work.tile([P, NT], F32, tag="lg")
        lsum = work.tile([P, 1], F32, tag="lsum")
        nc.scalar.activation(
            out=lg[:], in_=S[:, 0:NT], func=AF.Ln, accum_out=lsum[:]
        )
        val = work.tile([P, 1], F32, tag="val")
        nc.vector.tensor_sub(out=val[:], in0=lsum[:], in1=S[:, NT : NT + 1])

        # ---- partition reduction to a scalar: ones.T-weighted matmul
        pres = fin_psum.tile([1, 1], F32, tag="pres")
        nc.tensor.matmul(pres[:], val[:], ones[:], start=True, stop=True)
        outsb = work.tile([1, 1], F32, tag="outsb")
        nc.vector.tensor_copy(out=outsb[:], in_=pres[:])
        nc.sync.dma_start(out=out_ext, in_=outsb[:])


_NC_CACHE = None


def _get_nc():
    global _NC_CACHE
    if _NC_CACHE is None:
        _NC_CACHE = build_kernel()
    return _NC_CACHE


def kernel(**inputs) -> np.ndarray:
    a = np.ascontiguousarray(
        np.asarray(inputs["encoder_embedding1"], dtype=np.float32)
    )
    c = np.ascontiguousarray(
        np.asarray(inputs["encoder_embedding2"], dtype=np.float32)
    )
    assert a.shape == (B, D) and c.shape == (B, D)

    nc = _get_nc()
    in_maps = [
        {
            "a": a,
            "c": c[m * SH : (m + 1) * SH],
            "a_s": a[m * SH : (m + 1) * SH],
        }
        for m in range(M)
    ]
    res = run_bass_kernel_spmd(nc, in_maps, core_ids=list(range(M)))
    return np.float32(res.results[0]["out"][0, 0])


# revision 36
# speedup vs baseline: 1.4996x; 1.1625x over previous
"""AlignConLoss on 8 TRN2 NeuronCores.

loss = sum_j [ logsumexp_i sim[i,j] ] - sum_j sim[j,j]
with sim = l2norm(enc2) @ l2norm(enc1).T   (B=8192, D=256, T=1)

Distribution: the BxB similarity matrix is sharded row-wise (contrast rows,
enc2) across the 8 cores.  Every core receives the full anchor matrix (enc1)
in its own HBM, so anchor norms and the anchor transpose are computed locally
and the only collective is one small AllGather of per-core partial sums at
the end.

Per core:
  * enc2 shard and enc1 are cast f32->bf16 by gpsimd DMAs straight into
    SBUF in row-tile-major layout [128, tiles, 256].
  * row norms via one fused square+row-sum (scalar_tensor_tensor accum) per
    row tile; 1/sqrt as exp(-0.5*ln x) so only one ACT table set is used.
  * contraction-major operands (d on partitions) are built with TensorE
    transposes (batched 4 per PSUM tile) + one [128,512] DVE copy per batch.
  * main loop over 64 anchor j-tiles: 4 bf16 matmuls [128x128]@[128x512]
    accumulate sim into PSUM [128,1024]; one Exp activation (scale =
    1/||a_j|| per partition, fused accumulate) yields per-column partial
    sums.  The anchor pipeline is software-pipelined one slab ahead of the
    matmul consumer.
  * finale: AllGather of [128, 65] partials, local sum/log/subtract, and a
    [128,1]x[128,1] matmul folds partitions into the scalar loss.
"""

import numpy as np

import concourse.bass as bass
import concourse.mybir as mybir
import concourse.tile as tile
from concourse import bacc
from concourse.bass_utils import run_bass_kernel_spmd
from concourse.masks import make_identity

P = 128          # partitions
B = 8192         # batch (anchors = contrast = B)
D = 256          # embedding dim
M = 8            # cores
SH = B // M      # 1024 rows per shard/slab
ST = SH // P     # 8 row-tiles per slab
NT = B // P      # 64 j-tiles
DH = D // P      # 2 contraction chunks of 128
IC = 512         # moving-operand free-dim chunk

F32 = mybir.dt.float32
BF16 = mybir.dt.bfloat16
AF = mybir.ActivationFunctionType
ALU = mybir.AluOpType
AX = mybir.AxisListType

REPLICAS = [list(range(M))]

# Both Exp and Ln are used throughout; the default table-load pass puts them
# in different ACT table sets, which costs a ~1.3us table reload on every
# Exp<->Ln alternation.  Restrict both functions to the one set that holds
# them together so exactly one table load is ever emitted.  Set IDs are
# positional, so only membership is edited, never order.
_gat_orig = None


def _gat_shared_exp_ln(arch):
    tabs = dict(_gat_orig(arch))
    target = "natural_log_exp_and_others"
    if target in tabs:
        for name in tabs:
            if name != target:
                tabs[name] = tabs[name] - {AF.Exp, AF.Ln}
    return tabs


def _install_act_table_patch():
    global _gat_orig
    from concourse import bacc as _bacc_mod

    if _gat_orig is None:
        _gat_orig = _bacc_mod.get_activation_tables
        _bacc_mod.get_activation_tables = _gat_shared_exp_ln


def build_kernel() -> bacc.Bacc:
    _install_act_table_patch()
    nc = bacc.Bacc(
        "TRN2",
        target_bir_lowering=False,
        debug=False,
        num_devices=M,
    )
    a_ext = nc.dram_tensor("a", [B, D], F32, kind="ExternalInput").ap()
    c_ext = nc.dram_tensor("c", [SH, D], F32, kind="ExternalInput").ap()
    s_ext = nc.dram_tensor("a_s", [SH, D], F32, kind="ExternalInput").ap()
    out_ext = nc.dram_tensor("out", [1, 1], F32, kind="ExternalOutput").ap()

    with tile.TileContext(nc) as tc:
        _body(tc, nc, a_ext, c_ext, s_ext, out_ext)

    nc.compile()
    return nc


def _norms_stt(nc, scr, src, accum):
    """accum[:,0] = sum_d src*src (one fused DVE op); out value is dead."""
    sq = scr.tile([P, D], BF16, tag="sq", name="sq")
    nc.vector.scalar_tensor_tensor(
        out=sq[:],
        in0=src,
        scalar=1.0,
        in1=src,
        op0=ALU.mult,
        op1=ALU.mult,
        accum_out=accum,
    )


def _transpose_batch(nc, tr_psum, identB, src_nat, dst, h, g):
    """Transpose 4 [128,128] blocks (row-tiles 4g..4g+3, d-half h) of a
    natural-layout tile into dst[:, h, g*512:(g+1)*512] via one PSUM tile."""
    trps = tr_psum.tile([P, 4 * P], BF16, tag="trps", name="trps")
    for k in range(4):
        t = 4 * g + k
        nc.tensor.transpose(
            trps[:, k * P : (k + 1) * P],
            src_nat[:, t, h * P : (h + 1) * P],
            identB[:],
        )
    nc.vector.tensor_copy(
        out=dst[:, h, g * 4 * P : (g + 1) * 4 * P], in_=trps[:]
    )


def _body(tc, nc, a_ext, c_ext, s_ext, out_ext):
    with (
        tc.tile_pool(name="const", bufs=1) as const,
        tc.tile_pool(name="work", bufs=2) as work,
        tc.tile_pool(name="scr", bufs=3) as scr,
        tc.tile_pool(name="mm_psum", bufs=3, space="PSUM") as mm_psum,
        tc.tile_pool(name="tr_psum", bufs=2, space="PSUM") as tr_psum,
        tc.tile_pool(name="dram", bufs=1, space="DRAM") as dram,
    ):
        # ---- persistent SBUF tensors
        # aT[s][p, h, n] = bf16(a[s*SH + n, h*P + p]); one slab per shard so
        # matmuls only wait on the slab they read.
        aT = [
            const.tile([P, DH, SH], BF16, tag=f"aT{s}", name=f"aT{s}")
            for s in range(M)
        ]
        cnT = const.tile([P, DH, SH], BF16, tag="cnT")
        c_nat = const.tile([P, ST, D], BF16, tag="c_nat")
        cn_nat = const.tile([P, ST, D], BF16, tag="cn_nat")
        s_nat = const.tile([P, ST, D], BF16, tag="s_nat")
        cnorm2 = const.tile([P, ST], F32, tag="cnorm2")
        snorm2 = const.tile([P, ST], F32, tag="snorm2")
        lnc = const.tile([P, ST], F32, tag="lnc")
        lns = const.tile([P, ST], F32, tag="lns")
        rinv_c = const.tile([P, ST], F32, tag="rinv_c")
        rinv_s = const.tile([P, ST], F32, tag="rinv_s")
        anorm2 = const.tile([P, NT], F32, tag="anorm2")
        lna = const.tile([P, NT], F32, tag="lna")
        rinva = const.tile([P, NT], F32, tag="rinva")
        colpart = const.tile([P, NT], F32, tag="colpart")
        diagp = const.tile([P, ST], F32, tag="diagp")
        diagacc = const.tile([P, 1], F32, tag="diagacc")
        ones = const.tile([P, 1], F32, tag="ones")
        identB = const.tile([P, P], BF16, tag="identB")

        # slab staging for the anchor natural layout (two in flight)
        a_nat = [
            work.tile([P, ST, D], BF16, tag="a_nat", name=f"a_nat{s}")
            for s in range(M)
        ]

        # ---- DRAM buffers (final collective only)
        agp_in = dram.tile([P, NT + 1], F32, tag="agp_in")
        agp_out = dram.tile([M * P, NT + 1], F32, tag="agp_out")

        nc.vector.memset(ones[:], 1.0)
        make_identity(nc, identB[:])

        # ---- contrast shard + anchor shard natural layouts (cast DMAs)
        nc.gpsimd.dma_start(
            out=c_nat[:], in_=c_ext.rearrange("(t p) d -> p t d", p=P)
        )
        nc.gpsimd.dma_start(
            out=s_nat[:], in_=s_ext.rearrange("(t p) d -> p t d", p=P)
        )

        # ---- contrast norms and 1/sqrt factors
        for t in range(ST):
            _norms_stt(nc, scr, c_nat[:, t], cnorm2[:, t : t + 1])
            _norms_stt(nc, scr, s_nat[:, t], snorm2[:, t : t + 1])
        nc.scalar.activation(out=lnc[:], in_=cnorm2[:], func=AF.Ln)
        nc.scalar.activation(out=rinv_c[:], in_=lnc[:], func=AF.Exp, scale=-0.5)
        nc.scalar.activation(out=lns[:], in_=snorm2[:], func=AF.Ln)
        nc.scalar.activation(out=rinv_s[:], in_=lns[:], func=AF.Exp, scale=-0.5)

        # ---- normalize contrast rows, transpose to cnT
        for t in range(ST):
            nc.vector.tensor_scalar_mul(
                out=cn_nat[:, t], in0=c_nat[:, t], scalar1=rinv_c[:, t : t + 1]
            )
        for h in range(DH):
            for g in range(ST // 4):
                _transpose_batch(nc, tr_psum, identB, cn_nat, cnT, h, g)

        # ---- diagonal partials: sim[j,j] for this shard's j
        # diagp[p,t] = (1/||a_j||) * sum_d cn[j,d] * a_raw[j,d]
        for t in range(ST):
            sq3 = scr.tile([P, D], BF16, tag="sq")
            nc.vector.scalar_tensor_tensor(
                out=sq3[:],
                in0=cn_nat[:, t],
                scalar=rinv_s[:, t : t + 1],
                in1=s_nat[:, t],
                op0=ALU.mult,
                op1=ALU.mult,
                accum_out=diagp[:, t : t + 1],
            )
        nc.vector.reduce_sum(out=diagacc[:], in_=diagp[:], axis=AX.X)

        # ---- anchor slab pipeline + main loop, software-pipelined:
        # prep(slab s) is traced before main(slab s-1) so the PE stream
        # interleaves the next slab's transposes with the current matmuls.
        def prep_slab(s):
            nc.gpsimd.dma_start(
                out=a_nat[s][:],
                in_=a_ext[s * SH : (s + 1) * SH, :].rearrange(
                    "(t p) d -> p t d", p=P
                ),
            )
            for t in range(ST):
                _norms_stt(
                    nc, scr, a_nat[s][:, t],
                    anorm2[:, s * ST + t : s * ST + t + 1],
                )
            nc.scalar.activation(
                out=lna[:, s * ST : (s + 1) * ST],
                in_=anorm2[:, s * ST : (s + 1) * ST],
                func=AF.Ln,
            )
            nc.scalar.activation(
                out=rinva[:, s * ST : (s + 1) * ST],
                in_=lna[:, s * ST : (s + 1) * ST],
                func=AF.Exp,
                scale=-0.5,
            )
            for h in range(DH):
                for g in range(ST // 4):
                    _transpose_batch(nc, tr_psum, identB, a_nat[s], aT[s], h, g)

        def main_slab(s):
            for t in range(ST):
                jt = s * ST + t
                jloc = t * P
                ps = mm_psum.tile([P, 2 * IC], F32, tag="mmps", name="mmps")
                for h in range(DH):
                    w = aT[s][:, h, jloc : jloc + P]
                    first, last = h == 0, h == DH - 1
                    nc.tensor.matmul(
                        ps[:, 0:IC], w, cnT[:, h, 0:IC], start=first, stop=last
                    )
                    nc.tensor.matmul(
                        ps[:, IC : 2 * IC],
                        w,
                        cnT[:, h, IC : 2 * IC],
                        start=first,
                        stop=last,
                    )
                # exp in place on PSUM: only the accum (column sums) is used
                nc.scalar.activation(
                    out=ps[:],
                    in_=ps[:],
                    func=AF.Exp,
                    scale=rinva[:, jt : jt + 1],
                    accum_out=colpart[:, jt : jt + 1],
                )

        for s in range(M):
            prep_slab(s)
            if s >= 1:
                main_slab(s - 1)
        main_slab(M - 1)

        # ---- cross-core combine: AllGather [128, 65] -> [1024, 65]
        nc.sync.dma_start(out=agp_in[:, 0:NT], in_=colpart[:])
        nc.sync.dma_start(out=agp_in[:, NT : NT + 1], in_=diagacc[:])
        nc.gpsimd.collective_compute(
            "AllGather",
            ALU.bypass,
            replica_groups=REPLICAS,
            ins=[agp_in[:].opt()],
            outs=[agp_out[:].opt()],
        )
        gath = work.tile([P, M, NT + 1], F32, tag="gath")
        nc.sync.dma_start(
            out=gath[:], in_=agp_out[:].rearrange("(m p) f -> p m f", p=P)
        )
        S = work.tile([P, NT + 1], F32, tag="Ssum")
        nc.vector.reduce_sum(
            out=S[:], in_=gath[:].rearrange("p m f -> p f m"), axis=AX.X
        )
        lg = work.tile([P, NT], F32, tag="lg")
        lsum = work.tile([P, 1], F32, tag="lsum")
        nc.scalar.activation(
            out=lg[:], in_=S[:, 0:NT], func=AF.Ln, accum_out=lsum[:]
        )
        val = work.tile([P, 1], F32, tag="val")
        nc.vector.tensor_sub(out=val[:], in0=lsum[:], in1=S[:, NT : NT + 1])

        # ---- partition reduction to a scalar: ones-weighted matmul
        pres = tr_psum.tile([1, 1], F32, tag="trps", name="pres")
        nc.tensor.matmul(pres[:], val[:], ones[:], start=True, stop=True)
        outsb = work.tile([1, 1], F32, tag="outsb")
        nc.vector.tensor_copy(out=outsb[:], in_=pres[:])
        nc.sync.dma_start(out=out_ext, in_=outsb[:])


_NC_CACHE = None


def _get_nc():
    global _NC_CACHE
    if _NC_CACHE is None:
        _NC_CACHE = build_kernel()
    return _NC_CACHE


def kernel(**inputs) -> np.ndarray:
    a = np.ascontiguousarray(
        np.asarray(inputs["encoder_embedding1"], dtype=np.float32)
    )
    c = np.ascontiguousarray(
        np.asarray(inputs["encoder_embedding2"], dtype=np.float32)
    )
    assert a.shape == (B, D) and c.shape == (B, D)

    nc = _get_nc()
    in_maps = [
        {
            "a": a,
            "c": c[m * SH : (m + 1) * SH],
            "a_s": a[m * SH : (m + 1) * SH],
        }
        for m in range(M)
    ]
    res = run_bass_kernel_spmd(nc, in_maps, core_ids=list(range(M)))
    return np.float32(res.results[0]["out"][0, 0])
